# revision 1
# baseline (speedup 1.0000x reference)
"""Sparse attention (talking-heads + top-64) Trainium2 kernel, 8-core SPMD.

Sharding: (batch, query-block) across 8 cores. Core c handles batch c//4;
its batch-group index g=c%4 selects query-blocks [g, 7-g, 8+g, 15-g], one
per "slot"; slot s is compiled for the max J of its qb group so the SPMD
program is identical on every core.

Column grid per row: [16 mem keys | 112 pad | 2048 seq keys], slot widths
J = 128 + 512*(s+1) = [640, 1152, 1664, 2176].

Per slot: QK (split-bf16 3-term, per head) -> ACT evac -> DMA partition
shuffle to (i_sub, h) rows -> block-diag pre-mix matmul (fp32) + additive
causal/pad mask matmuls -> ACT evac -> DRAM dm buffer. Per row-group g:
8-round max8/match_replace top-64 (DVE) -> denominator from the 64
extracted values -> ACT exp with folded -max-ln(den) bias -> DVE is_ge
mask + GP multiply -> p~ bf16 -> fused post-mix+transpose matmul (bf16)
-> pT -> AV (bf16) -> Wo projection (bf16) + bias -> out.
"""
import numpy as np
import ml_dtypes

bf16 = ml_dtypes.bfloat16

B, N, DIM = 2, 2048, 1024
H, D, M = 16, 64, 16
TOPK = 64
NEG = -1.0e30
SLOT_J = [640, 1152, 1664, 2176]
NSLOT = 4
CW = 256  # QK/mix j-chunk width
QBS_OF_G = [[g, 7 - g, 8 + g, 15 - g] for g in range(4)]

_nc_cache = {}


def _split_hi_lo(a):
    hi = a.astype(bf16)
    lo = (a.astype(np.float32) - hi.astype(np.float32)).astype(bf16)
    return hi, lo


def _chunk_part(a):
    """[K, F] -> [128, K//128, F] with partition = K % 128 within chunk."""
    K, F = a.shape
    return np.ascontiguousarray(a.reshape(K // 128, 128, F).transpose(1, 0, 2))


def build_nc(reps=1):
    import concourse.bass as bass
    import concourse.tile as tile
    import concourse.mybir as mybir
    from concourse import bacc

    dt = mybir.dt
    nc = bacc.Bacc()

    def din(name, shape, d=dt.bfloat16):
        return nc.dram_tensor(name, shape, d, kind="ExternalInput")

    xT_hi = din("xT_hi", [128, 8, N])
    xT_lo = din("xT_lo", [128, 8, N])
    xqT_hi = din("xqT_hi", [128, 8, NSLOT, 128])
    xqT_lo = din("xqT_lo", [128, 8, NSLOT, 128])
    wq_hi = din("wq_hi", [128, 8, DIM])
    wq_lo = din("wq_lo", [128, 8, DIM])
    wk_hi = din("wk_hi", [128, 8, DIM])
    wk_lo = din("wk_lo", [128, 8, DIM])
    wv = din("wv", [128, 8, DIM])
    wo = din("wo", [128, 8, DIM])
    memKT_hi = din("memKT_hi", [128, 8, 128])
    memKT_lo = din("memKT_lo", [128, 8, 128])
    memV = din("memV", [128, DIM])
    w1 = din("w1", [128, 128], dt.float32)
    w2 = din("w2", [128, 128])
    selg = din("selg", [128, 16, 128], dt.float32)
    ones1 = din("ones1", [1, 128], dt.float32)
    padrow = din("padrow", [1, CW], dt.float32)
    masks = din("masks", [128, NSLOT, 640], dt.float32)
    bo_in = din("bo_in", [128, DIM], dt.float32)

    out_d = nc.dram_tensor("out", [NSLOT, 128, DIM], dt.float32,
                           kind="ExternalOutput")

    with tile.TileContext(nc) as tc:
      for _rep in range(reps):
        with tc.tile_pool(name="persist", bufs=1) as pool_w, \
             tc.tile_pool(name="dscratch", bufs=1, space="DRAM") as pool_dram, \
             tc.tile_pool(name="dots", bufs=1) as pool_dots, \
             tc.tile_pool(name="kts", bufs=2) as pool_kts, \
             tc.tile_pool(name="shuf", bufs=3) as pool_shuf, \
             tc.tile_pool(name="dmc", bufs=3) as pool_dmc, \
             tc.tile_pool(name="psD", bufs=2, space="PSUM") as pool_psd, \
             tc.tile_pool(name="psMix", bufs=2, space="PSUM") as pool_psmx:

            wo_s = pool_w.tile([128, 8, DIM], dt.bfloat16)
            w1_s = pool_w.tile([128, 128], dt.float32)
            w2_s = pool_w.tile([128, 128], dt.bfloat16)
            selg_s = pool_w.tile([128, 16, 128], dt.float32)
            ones1_s = pool_w.tile([1, 128], dt.float32)
            padrow_s = pool_w.tile([1, CW], dt.float32)
            masks_s = pool_w.tile([128, NSLOT, 640], dt.float32)
            bo_s = pool_w.tile([128, DIM], dt.float32)
            qt_hi = pool_w.tile([128, NSLOT, 8, 128], dt.bfloat16)
            qt_lo = pool_w.tile([128, NSLOT, 8, 128], dt.bfloat16)
            for c in range(8):
                nc.sync.dma_start(wo_s[:, c, :], wo[:, c, :])
            nc.sync.dma_start(w1_s[:], w1[:])
            nc.sync.dma_start(w2_s[:], w2[:])
            nc.sync.dma_start(selg_s[:], selg[:])
            nc.sync.dma_start(ones1_s[:], ones1[:])
            nc.sync.dma_start(padrow_s[:], padrow[:])
            nc.sync.dma_start(masks_s[:], masks[:])
            nc.sync.dma_start(bo_s[:], bo_in[:])

            kt_hi_d = pool_dram.tile([128, 8, 2176], dt.bfloat16)
            kt_lo_d = pool_dram.tile([128, 8, 2176], dt.bfloat16)
            v_d = pool_dram.tile([128, 16, 17, 64], dt.bfloat16)

            def do_jc(s, J, NJC, NJP, dm_dram, jc):
                W = CW if jc < NJC - 1 else 128
                j0 = jc * CW
                kth = pool_kts.tile([128, 8, CW], dt.bfloat16, tag="kth")
                ktl = pool_kts.tile([128, 8, CW], dt.bfloat16, tag="ktl")
                nc.sync.dma_start(kth[:, :, 0:W], kt_hi_d[:, :, j0:j0 + W])
                nc.sync.dma_start(ktl[:, :, 0:W], kt_lo_d[:, :, j0:j0 + W])
                dots = pool_dots.tile([128, 16, CW], dt.float32, tag="dots")
                for h in range(16):
                    pb = (h % 2) * 64
                    pl = h // 2
                    psd = pool_psd.tile([128, CW], dt.float32, tag="psd")
                    lhi = qt_hi[pb:pb + 64, s, pl, :]
                    llo = qt_lo[pb:pb + 64, s, pl, :]
                    rhi = kth[pb:pb + 64, pl, 0:W]
                    rlo = ktl[pb:pb + 64, pl, 0:W]
                    nc.tensor.matmul(psd[:, 0:W], lhi, rhi, start=True,
                                     stop=False)
                    nc.tensor.matmul(psd[:, 0:W], lhi, rlo, start=False,
                                     stop=False)
                    nc.tensor.matmul(psd[:, 0:W], llo, rhi, start=False,
                                     stop=True)
                    nc.scalar.copy(dots[:, h, 0:W], psd[:, 0:W])
                mwin0 = J - 640
                dmc = None
                for g in range(16):
                    shuf = pool_shuf.tile([128, CW], dt.float32, tag="shuf")
                    nc.gpsimd.dma_start(
                        shuf[:, 0:W], dots[g * 8:(g + 1) * 8, :, 0:W])
                    psm = pool_psmx.tile([128, CW], dt.float32, tag="psm")
                    mm = [(w1_s[:], shuf[:, 0:W], slice(0, W))]
                    if jc == 0:
                        mm.append((ones1_s[:], padrow_s[:, 0:W], slice(0, W)))
                    if j0 + W > mwin0:
                        a = max(j0, mwin0)
                        mm.append((selg_s[:, g, :],
                                   masks_s[:, s, a - mwin0:j0 + W - mwin0],
                                   slice(a - j0, W)))
                    for mi, (lh, rh, csl) in enumerate(mm):
                        nc.tensor.matmul(psm[:, csl], lh, rh,
                                         start=(mi == 0),
                                         stop=(mi == len(mm) - 1))
                    if g % 4 == 0:
                        dmc = pool_dmc.tile([128, 4, CW], dt.float32,
                                            tag="dmc")
                    nc.scalar.copy(dmc[:, g % 4, 0:W], psm[:, 0:W])
                    if g % 4 == 3:
                        nc.sync.dma_start(
                            dm_dram[g - 3:g + 1, :, j0:j0 + W]
                            .rearrange("g p j -> p g j"), dmc[:, :, 0:W])

            dm_s0 = pool_dram.tile([16, 128, 2176], dt.float32, tag="dm0")

            # ============ phase 1: QT, K^T (slot-0 jc interleaved), V ======
            with tc.tile_pool(name="ph1", bufs=1) as p1, \
                 tc.tile_pool(name="ph1c", bufs=3) as p1c, \
                 tc.tile_pool(name="ph1x", bufs=2) as p1x, \
                 tc.tile_pool(name="ph1q", bufs=2) as p1q, \
                 tc.tile_pool(name="ps1", bufs=2, space="PSUM") as ps1:
                wv_s = p1.tile([128, 8, DIM], dt.bfloat16)
                xq_hi_s = p1.tile([128, 8, NSLOT, 128], dt.bfloat16)
                xq_lo_s = p1.tile([128, 8, NSLOT, 128], dt.bfloat16)
                for c in range(8):
                    nc.sync.dma_start(wv_s[:, c, :], wv[:, c, :])
                nc.sync.dma_start(xq_hi_s[:], xqT_hi[:])
                nc.sync.dma_start(xq_lo_s[:], xqT_lo[:])
                for c in range(8):
                    nc.sync.dma_start(kt_hi_d[:, c, 0:128], memKT_hi[:, c, :])
                    nc.sync.dma_start(kt_lo_d[:, c, 0:128], memKT_lo[:, c, :])
                nc.sync.dma_start(v_d[:, :, 0, :], memV[:])

                # QT (wq streamed per m-chunk)
                for mc in range(8):
                    wqc_h = p1q.tile([128, 8, 128], dt.bfloat16, tag="wqch")
                    wqc_l = p1q.tile([128, 8, 128], dt.bfloat16, tag="wqcl")
                    nc.sync.dma_start(wqc_h[:],
                                      wq_hi[:, :, mc * 128:(mc + 1) * 128])
                    nc.sync.dma_start(wqc_l[:],
                                      wq_lo[:, :, mc * 128:(mc + 1) * 128])
                    for s in range(NSLOT):
                        psq = ps1.tile([128, 128], dt.float32, tag="psq")
                        for dc in range(8):
                            nc.tensor.matmul(psq[:], wqc_h[:, dc, :],
                                             xq_hi_s[:, dc, s, :],
                                             start=(dc == 0), stop=False)
                            nc.tensor.matmul(psq[:], wqc_h[:, dc, :],
                                             xq_lo_s[:, dc, s, :],
                                             start=False, stop=False)
                            nc.tensor.matmul(psq[:], wqc_l[:, dc, :],
                                             xq_hi_s[:, dc, s, :],
                                             start=False, stop=(dc == 7))
                        qt32 = p1q.tile([128, 128], dt.float32, tag="qt32")
                        nc.scalar.copy(qt32[:], psq[:])
                        nc.vector.tensor_copy(qt_hi[:, s, mc, :], qt32[:])
                        nc.vector.tensor_sub(qt_lo[:, s, mc, :], qt32[:],
                                             qt_hi[:, s, mc, :])

                def do_kt(njc):
                    xth = p1x.tile([128, 8, 512], dt.bfloat16, tag="xth")
                    xtl = p1x.tile([128, 8, 512], dt.bfloat16, tag="xtl")
                    nc.sync.dma_start(
                        xth[:], xT_hi[:, :, njc * 512:(njc + 1) * 512])
                    nc.sync.dma_start(
                        xtl[:], xT_lo[:, :, njc * 512:(njc + 1) * 512])
                    for ic in range(8):
                        wkc_h = p1q.tile([128, 8, 128], dt.bfloat16,
                                         tag="wkch")
                        wkc_l = p1q.tile([128, 8, 128], dt.bfloat16,
                                         tag="wkcl")
                        nc.sync.dma_start(
                            wkc_h[:], wk_hi[:, :, ic * 128:(ic + 1) * 128])
                        nc.sync.dma_start(
                            wkc_l[:], wk_lo[:, :, ic * 128:(ic + 1) * 128])
                        ps = ps1.tile([128, 512], dt.float32, tag="psk")
                        for dc in range(8):
                            nc.tensor.matmul(ps[:], wkc_h[:, dc, :],
                                             xth[:, dc, :], start=(dc == 0),
                                             stop=False)
                            nc.tensor.matmul(ps[:], wkc_h[:, dc, :],
                                             xtl[:, dc, :], start=False,
                                             stop=False)
                            nc.tensor.matmul(ps[:], wkc_l[:, dc, :],
                                             xth[:, dc, :], start=False,
                                             stop=(dc == 7))
                        khi = p1c.tile([128, 512], dt.bfloat16, tag="khi")
                        klo = p1c.tile([128, 512], dt.bfloat16, tag="klo")
                        nc.scalar.copy(khi[:], ps[:])
                        nc.vector.tensor_sub(klo[:], ps[:], khi[:])
                        cols = slice(128 + njc * 512, 128 + (njc + 1) * 512)
                        nc.sync.dma_start(kt_hi_d[:, ic, cols], khi[:])
                        nc.sync.dma_start(kt_lo_d[:, ic, cols], klo[:])

                do_kt(0)
                # slot-0 QK/mix pipeline interleaves with remaining phase-1
                for jc in range(3):
                    do_jc(0, SLOT_J[0], 3, 5, dm_s0, jc)
                for njc in range(1, 4):
                    do_kt(njc)

                for nb in range(16):
                    xthv = p1x.tile([128, 8, 128], dt.bfloat16, tag="xthv")
                    nc.sync.dma_start(
                        xthv[:], xT_hi[:, :, nb * 128:(nb + 1) * 128])
                    for fh in range(2):
                        ps = ps1.tile([128, 512], dt.float32, tag="psk")
                        for dc in range(8):
                            nc.tensor.matmul(
                                ps[:], xthv[:, dc, :],
                                wv_s[:, dc, fh * 512:(fh + 1) * 512],
                                start=(dc == 0), stop=(dc == 7))
                        vsb = p1c.tile([128, 512], dt.bfloat16, tag="vsb")
                        nc.scalar.copy(vsb[:], ps[:])
                        nc.sync.dma_start(
                            v_d[:, fh * 8:(fh + 1) * 8, 1 + nb, :], vsb[:])

            # ============ phase 2: attention =============================
            with tc.tile_pool(name="dmg", bufs=2) as pool_dmg, \
                 tc.tile_pool(name="tk1", bufs=1) as pool_tk1, \
                 tc.tile_pool(name="tk2", bufs=2) as pool_tk2, \
                 tc.tile_pool(name="sm", bufs=4) as pool_sm, \
                 tc.tile_pool(name="ptq", bufs=1) as pool_ptq, \
                 tc.tile_pool(name="vs", bufs=2) as pool_vs, \
                 tc.tile_pool(name="outs", bufs=1) as pool_out, \
                 tc.tile_pool(name="psPt", bufs=2, space="PSUM") as pool_pspt, \
                 tc.tile_pool(name="psAvO", bufs=2, space="PSUM") as pool_psav:

                def do_gq(s, J, NJC, NJP, dm_dram, av32, gq):
                    ptq = pool_ptq.tile([128, 17, 16, 32], dt.bfloat16,
                                        tag="ptq")
                    for gi in range(4):
                        g = gq * 4 + gi
                        dmg = pool_dmg.tile([128, 2176], dt.float32,
                                            tag="dmg")
                        nc.sync.dma_start(dmg[:, 0:J], dm_dram[g, :, 0:J])
                        scr = pool_tk1.tile([128, 2176], dt.float32,
                                            tag="scr")
                        cands = pool_tk2.tile([128, 64], dt.float32,
                                              tag="cands")
                        if s == 0:
                            nc.vector.max(cands[:, 0:8], dmg[:, 0:J])
                            nc.vector.match_replace(scr[:, 0:J],
                                                    cands[:, 0:8],
                                                    dmg[:, 0:J], NEG)
                            for r in range(1, 8):
                                nc.vector.max(cands[:, r * 8:(r + 1) * 8],
                                              scr[:, 0:J])
                                if r < 7:
                                    nc.vector.match_replace(
                                        scr[:, 0:J],
                                        cands[:, r * 8:(r + 1) * 8],
                                        scr[:, 0:J], NEG)
                        else:
                            RR = 4 if s == 1 else 3
                            segw = (J - 128) // 8
                            segs = [(0, 128)] + [
                                (128 + k * segw, 128 + (k + 1) * segw)
                                for k in range(8)]
                            cpool = pool_tk2.tile([128, 9 * 8 * 4],
                                                  dt.float32, tag="cpool")
                            for rr in range(RR):
                                src_t = dmg if rr == 0 else scr
                                for si2, (a2, b2) in enumerate(segs):
                                    nc.vector.max(
                                        cpool[:, (rr * 9 + si2) * 8:
                                              (rr * 9 + si2) * 8 + 8],
                                        src_t[:, a2:b2])
                                if rr < RR - 1:
                                    for si2, (a2, b2) in enumerate(segs):
                                        nc.vector.match_replace(
                                            scr[:, a2:b2],
                                            cpool[:, (rr * 9 + si2) * 8:
                                                  (rr * 9 + si2) * 8 + 8],
                                            src_t[:, a2:b2], NEG)
                            ncand = 9 * 8 * RR
                            nc.vector.max(cands[:, 0:8], cpool[:, 0:ncand])
                            nc.vector.match_replace(cpool[:, 0:ncand],
                                                    cands[:, 0:8],
                                                    cpool[:, 0:ncand], NEG)
                            for r in range(1, 8):
                                nc.vector.max(cands[:, r * 8:(r + 1) * 8],
                                              cpool[:, 0:ncand])
                                if r < 7:
                                    nc.vector.match_replace(
                                        cpool[:, 0:ncand],
                                        cands[:, r * 8:(r + 1) * 8],
                                        cpool[:, 0:ncand], NEG)
                        negm = pool_sm.tile([128, 1], dt.float32, tag="negm")
                        nc.vector.tensor_scalar_mul(negm[:], cands[:, 0:1],
                                                    -1.0)
                        ec = pool_sm.tile([128, 64], dt.float32, tag="ec")
                        nc.scalar.activation(ec[:], cands[:],
                                             mybir.ActivationFunctionType.Exp,
                                             bias=negm[:])
                        den = pool_sm.tile([128, 1], dt.float32, tag="den")
                        nc.vector.reduce_sum(den[:], ec[:],
                                             axis=mybir.AxisListType.X)
                        rden = pool_sm.tile([128, 1], dt.float32, tag="rden")
                        nc.vector.reciprocal(rden[:], den[:])
                        p2 = pool_tk2.tile([128, 2176], dt.float32, tag="p2")
                        nc.scalar.activation(p2[:, 0:J], dmg[:, 0:J],
                                             mybir.ActivationFunctionType.Exp,
                                             bias=negm[:])
                        nc.vector.tensor_scalar(scr[:, 0:J], dmg[:, 0:J],
                                                cands[:, 63:64], rden[:],
                                                mybir.AluOpType.is_ge,
                                                mybir.AluOpType.mult)
                        ptil = pool_tk2.tile([128, 2176], dt.bfloat16,
                                             tag="ptil")
                        nc.gpsimd.tensor_mul(ptil[:, 0:J], scr[:, 0:J],
                                             p2[:, 0:J])
                        for jp4 in range((NJP + 3) // 4):
                            nq = min(4, NJP - jp4 * 4)
                            pspt = pool_pspt.tile([128, 512], dt.float32,
                                                  tag="pspt")
                            for q in range(nq):
                                jp = jp4 * 4 + q
                                nc.tensor.matmul(
                                    pspt[:, q * 128:(q + 1) * 128],
                                    ptil[:, jp * 128:(jp + 1) * 128],
                                    w2_s[:], start=True, stop=True)
                            dst = ptq[:, jp4 * 4:jp4 * 4 + nq, :,
                                      gi * 8:(gi + 1) * 8]
                            nc.scalar.copy(dst[:], pspt[:, 0:nq * 128])
                    # AV over this quad (32 query columns)
                    psav = pool_psav.tile([128, 8, 32], dt.float32,
                                          tag="avo")
                    for ko in range(16):
                        vst = pool_vs.tile([128, 17, 64], dt.bfloat16,
                                           tag="vst")
                        nc.gpsimd.dma_start(vst[:, 0:NJP, :],
                                            v_d[:, ko, 0:NJP, :])
                        pb = (ko % 2) * 64
                        for jp in range(NJP):
                            nc.tensor.matmul(
                                psav[pb:pb + 64, ko // 2, :],
                                vst[:, jp, :], ptq[:, jp, ko, :],
                                start=(jp == 0), stop=(jp == NJP - 1))
                    nc.vector.tensor_copy(av32[:, :, gq * 32:(gq + 1) * 32],
                                          psav[:])

                def do_tail(s, J, av32):
                    av_sb = pool_out.tile([128, 8, 128], dt.bfloat16,
                                          tag="av")
                    nc.vector.tensor_copy(av_sb[:], av32[:])
                    osb = pool_out.tile([128, DIM], dt.float32, tag="osb")
                    for fh in range(2):
                        pso = pool_psav.tile([128, 512], dt.float32,
                                             tag="avo")
                        for cp in range(8):
                            nc.tensor.matmul(
                                pso[:], av_sb[:, cp, :],
                                wo_s[:, cp, fh * 512:(fh + 1) * 512],
                                start=(cp == 0), stop=(cp == 7))
                        nc.vector.tensor_add(osb[:, fh * 512:(fh + 1) * 512],
                                             pso[:],
                                             bo_s[:, fh * 512:(fh + 1) * 512])
                    nc.sync.dma_start(out_d[s, :, :], osb[:])

                for s in range(NSLOT):
                    J = SLOT_J[s]
                    NJC = (J - 128) // CW + 1
                    NJP = J // 128
                    if s == 0:
                        dm_dram = dm_s0
                    else:
                        dm_dram = pool_dram.tile([16, 128, 2176], dt.float32,
                                                 tag=f"dm{s % 2}")
                        for jc in range(NJC):
                            do_jc(s, J, NJC, NJP, dm_dram, jc)
                    av32 = pool_out.tile([128, 8, 128], dt.float32,
                                         tag="av32")
                    for gq in range(4):
                        do_gq(s, J, NJC, NJP, dm_dram, av32, gq)
                    do_tail(s, J, av32)

    nc.finalize()
    return nc


_prep_cache = {}


def _host_prep(core, inputs):
    x = np.asarray(inputs["x"], dtype=np.float32)
    Wq = np.asarray(inputs["Wq"], dtype=np.float32) * (D ** -0.5)
    Wk = np.asarray(inputs["Wk"], dtype=np.float32)
    Wv = np.asarray(inputs["Wv"], dtype=np.float32)
    Wo = np.asarray(inputs["Wo"], dtype=np.float32)
    bo = np.asarray(inputs["bo"], dtype=np.float32)
    pre = np.asarray(inputs["pre_proj"], dtype=np.float32)
    post = np.asarray(inputs["post_proj"], dtype=np.float32)
    mem_k = np.asarray(inputs["mem_k"], dtype=np.float32)
    mem_v = np.asarray(inputs["mem_v"], dtype=np.float32)

    b = core // 4
    g = core % 4
    qbs = QBS_OF_G[g]

    xb = x[b]
    if ("xT", b) not in _prep_cache:
        xT = np.ascontiguousarray(xb.T)
        _prep_cache[("xT", b)] = _split_hi_lo(xT)
    xT_hi, xT_lo = _prep_cache[("xT", b)]
    xq = np.concatenate([xb[qb * 128:(qb + 1) * 128] for qb in qbs], axis=0)
    xqT = np.ascontiguousarray(xq.T)                      # [DIM, 512]
    xqT_hi, xqT_lo = _split_hi_lo(xqT)

    if "w" not in _prep_cache:
        _prep_cache["w"] = (_split_hi_lo(Wq), _split_hi_lo(Wk))
    (wq_hi, wq_lo), (wk_hi, wk_lo) = _prep_cache["w"]

    mkt = np.zeros((128, 8, 128), dtype=np.float32)
    for h in range(H):
        mkt[(h % 2) * 64:(h % 2) * 64 + 64, h // 2, 0:M] = mem_k[h].T
    mkt_hi, mkt_lo = _split_hi_lo(mkt)
    mv = np.zeros((128, DIM), dtype=np.float32)
    mv[0:M] = mem_v.transpose(1, 0, 2).reshape(M, DIM)

    w1 = np.zeros((128, 128), dtype=np.float32)
    for isub in range(8):
        for h in range(H):
            for k in range(H):
                w1[isub * 16 + h, k * 8 + isub] = pre[h, k]
    w2 = np.zeros((128, 128), dtype=np.float32)
    for isub in range(8):
        for k in range(H):
            for ko in range(H):
                w2[k * 8 + isub, ko * 8 + isub] = post[k, ko]
    selg = np.zeros((128, 16, 128), dtype=np.float32)
    for gg in range(16):
        for isub in range(8):
            selg[gg * 8 + isub, gg, np.arange(16) * 8 + isub] = 1.0
    ones1 = np.ones((1, 128), dtype=np.float32)
    padrow = np.zeros((1, CW), dtype=np.float32)
    padrow[0, M:128] = NEG

    masks = np.zeros((128, NSLOT, 640), dtype=np.float32)
    for si, qb in enumerate(qbs):
        J = SLOT_J[si]
        base = J - 640
        for gg in range(16):
            for isub in range(8):
                i_glob = qb * 128 + gg * 8 + isub
                jmax = 128 + i_glob + 1
                cols = np.arange(base, J)
                masks[gg * 8 + isub, si, cols >= jmax] = NEG

    wo_r = np.zeros((128, 8, DIM), dtype=np.float32)
    for ko in range(H):
        wo_r[(ko % 2) * 64:(ko % 2) * 64 + 64, ko // 2, :] = \
            Wo[ko * 64:(ko + 1) * 64, :]

    f = np.ascontiguousarray
    return {
        "xT_hi": f(_chunk_part(xT_hi)), "xT_lo": f(_chunk_part(xT_lo)),
        "xqT_hi": f(_chunk_part(xqT_hi).reshape(128, 8, NSLOT, 128)),
        "xqT_lo": f(_chunk_part(xqT_lo).reshape(128, 8, NSLOT, 128)),
        "wq_hi": f(_chunk_part(wq_hi)), "wq_lo": f(_chunk_part(wq_lo)),
        "wk_hi": f(_chunk_part(wk_hi)), "wk_lo": f(_chunk_part(wk_lo)),
        "wv": f(_chunk_part(Wv.astype(bf16))),
        "wo": f(wo_r.astype(bf16)),
        "memKT_hi": f(mkt_hi), "memKT_lo": f(mkt_lo),
        "memV": f(mv.astype(bf16)),
        "w1": w1, "w2": f(w2.astype(bf16)), "selg": selg, "ones1": ones1,
        "padrow": padrow, "masks": masks,
        "bo_in": f(np.broadcast_to(bo[None, :], (128, DIM)).copy()),
    }


def kernel(**inputs) -> np.ndarray:
    from concourse.bass_utils import run_bass_kernel_spmd

    _prep_cache.clear()

    if "nc" not in _nc_cache:
        _nc_cache["nc"] = build_nc()
    nc = _nc_cache["nc"]

    in_maps = [_host_prep(c, inputs) for c in range(8)]
    res = run_bass_kernel_spmd(nc, in_maps, core_ids=list(range(8)))

    out = np.zeros((B, N, DIM), dtype=np.float32)
    for c in range(8):
        b = c // 4
        qbs = QBS_OF_G[c % 4]
        o = res.results[c]["out"]
        for si, qb in enumerate(qbs):
            out[b, qb * 128:(qb + 1) * 128, :] = o[si]
    return out



# revision 2
# speedup vs baseline: 1088.9083x; 1088.9083x over previous
"""Sparse attention (talking-heads + top-64) Trainium2 kernel, 8-core SPMD.

Sharding: (batch, query-block) across 8 cores. Core c handles batch c//4;
its batch-group index g=c%4 selects query-blocks [g, 7-g, 8+g, 15-g], one
per "slot"; slot s is compiled for the max J of its qb group so the SPMD
program is identical on every core.

Column grid per row: [16 mem keys | 112 pad | 2048 seq keys], slot widths
J = 128 + 512*(s+1) = [640, 1152, 1664, 2176].

Per slot: QK (split-bf16 3-term, per head) -> ACT evac -> DMA partition
shuffle to (i_sub, h) rows -> block-diag pre-mix matmul (fp32) + additive
causal/pad mask matmuls -> ACT evac -> DRAM dm buffer. Per row-group g:
8-round max8/match_replace top-64 (DVE) -> denominator from the 64
extracted values -> ACT exp with folded -max-ln(den) bias -> DVE is_ge
mask + GP multiply -> p~ bf16 -> fused post-mix+transpose matmul (bf16)
-> pT -> AV (bf16) -> Wo projection (bf16) + bias -> out.
"""
import numpy as np
import ml_dtypes

bf16 = ml_dtypes.bfloat16

B, N, DIM = 2, 2048, 1024
H, D, M = 16, 64, 16
TOPK = 64
NEG = -1.0e30
SLOT_J = [640, 1152, 1664, 2176]
NSLOT = 4
CW = 256  # QK/mix j-chunk width
QBS_OF_G = [[g, 7 - g, 8 + g, 15 - g] for g in range(4)]

_nc_cache = {}


def _split_hi_lo(a):
    hi = a.astype(bf16)
    lo = (a.astype(np.float32) - hi.astype(np.float32)).astype(bf16)
    return hi, lo


def _chunk_part(a):
    """[K, F] -> [128, K//128, F] with partition = K % 128 within chunk."""
    K, F = a.shape
    return np.ascontiguousarray(a.reshape(K // 128, 128, F).transpose(1, 0, 2))


def build_nc(reps=1):
    import concourse.bass as bass
    import concourse.tile as tile
    import concourse.mybir as mybir
    from concourse import bacc

    dt = mybir.dt
    nc = bacc.Bacc()

    def din(name, shape, d=dt.bfloat16):
        return nc.dram_tensor(name, shape, d, kind="ExternalInput")

    xT_hi = din("xT_hi", [128, 8, N])
    xT_lo = din("xT_lo", [128, 8, N])
    xqT_hi = din("xqT_hi", [128, 8, NSLOT, 128])
    xqT_lo = din("xqT_lo", [128, 8, NSLOT, 128])
    wq_hi = din("wq_hi", [128, 8, DIM])
    wq_lo = din("wq_lo", [128, 8, DIM])
    wk_hi = din("wk_hi", [128, 8, DIM])
    wk_lo = din("wk_lo", [128, 8, DIM])
    wv = din("wv", [128, 8, DIM])
    wo = din("wo", [128, 8, DIM])
    memKT_hi = din("memKT_hi", [128, 8, 128])
    memKT_lo = din("memKT_lo", [128, 8, 128])
    memV = din("memV", [128, DIM])
    w1 = din("w1", [128, 128], dt.float32)
    w2 = din("w2", [128, 128])
    selg = din("selg", [128, 16, 128], dt.float32)
    ones1 = din("ones1", [1, 128], dt.float32)
    padrow = din("padrow", [1, CW], dt.float32)
    masks = din("masks", [128, NSLOT, 640], dt.float32)
    bo_in = din("bo_in", [128, DIM], dt.float32)

    out_d = nc.dram_tensor("out", [NSLOT, 128, DIM], dt.float32,
                           kind="ExternalOutput")

    with tile.TileContext(nc) as tc:
      for _rep in range(reps):
        with tc.tile_pool(name="persist", bufs=1) as pool_w, \
             tc.tile_pool(name="dscratch", bufs=1, space="DRAM") as pool_dram, \
             tc.tile_pool(name="dots", bufs=1) as pool_dots, \
             tc.tile_pool(name="kts", bufs=2) as pool_kts, \
             tc.tile_pool(name="shuf", bufs=3) as pool_shuf, \
             tc.tile_pool(name="dmc", bufs=3) as pool_dmc, \
             tc.tile_pool(name="psD", bufs=2, space="PSUM") as pool_psd, \
             tc.tile_pool(name="psMix", bufs=2, space="PSUM") as pool_psmx:

            wo_s = pool_w.tile([128, 8, DIM], dt.bfloat16)
            w1_s = pool_w.tile([128, 128], dt.float32)
            w2_s = pool_w.tile([128, 128], dt.bfloat16)
            selg_s = pool_w.tile([128, 16, 128], dt.float32)
            ones1_s = pool_w.tile([1, 128], dt.float32)
            padrow_s = pool_w.tile([1, CW], dt.float32)
            masks_s = pool_w.tile([128, NSLOT, 640], dt.float32)
            bo_s = pool_w.tile([128, DIM], dt.float32)
            qt_hi = pool_w.tile([128, NSLOT, 8, 128], dt.bfloat16)
            qt_lo = pool_w.tile([128, NSLOT, 8, 128], dt.bfloat16)
            for c in range(8):
                nc.sync.dma_start(wo_s[:, c, :], wo[:, c, :])
            nc.sync.dma_start(w1_s[:], w1[:])
            nc.sync.dma_start(w2_s[:], w2[:])
            nc.sync.dma_start(selg_s[:], selg[:])
            nc.sync.dma_start(ones1_s[:], ones1[:])
            nc.sync.dma_start(padrow_s[:], padrow[:])
            nc.sync.dma_start(masks_s[:], masks[:])
            nc.sync.dma_start(bo_s[:], bo_in[:])

            kt_hi_d = pool_dram.tile([128, 8, 2176], dt.bfloat16)
            kt_lo_d = pool_dram.tile([128, 8, 2176], dt.bfloat16)
            v_d = pool_dram.tile([128, 16, 17, 64], dt.bfloat16)

            def do_jc(s, J, NJC, NJP, dm_dram, jc):
                W = CW if jc < NJC - 1 else 128
                j0 = jc * CW
                kth = pool_kts.tile([128, 8, CW], dt.bfloat16, tag="kth")
                ktl = pool_kts.tile([128, 8, CW], dt.bfloat16, tag="ktl")
                nc.sync.dma_start(kth[:, :, 0:W], kt_hi_d[:, :, j0:j0 + W])
                nc.sync.dma_start(ktl[:, :, 0:W], kt_lo_d[:, :, j0:j0 + W])
                dots = pool_dots.tile([128, 16, CW], dt.float32, tag="dots")
                for h in range(16):
                    pb = (h % 2) * 64
                    pl = h // 2
                    psd = pool_psd.tile([128, CW], dt.float32, tag="psd")
                    lhi = qt_hi[pb:pb + 64, s, pl, :]
                    llo = qt_lo[pb:pb + 64, s, pl, :]
                    rhi = kth[pb:pb + 64, pl, 0:W]
                    rlo = ktl[pb:pb + 64, pl, 0:W]
                    nc.tensor.matmul(psd[:, 0:W], lhi, rhi, start=True,
                                     stop=False)
                    nc.tensor.matmul(psd[:, 0:W], lhi, rlo, start=False,
                                     stop=False)
                    nc.tensor.matmul(psd[:, 0:W], llo, rhi, start=False,
                                     stop=True)
                    nc.scalar.copy(dots[:, h, 0:W], psd[:, 0:W])
                mwin0 = J - 640
                dmc = None
                for g in range(16):
                    shuf = pool_shuf.tile([128, CW], dt.float32, tag="shuf")
                    nc.gpsimd.dma_start(
                        shuf[:, 0:W], dots[g * 8:(g + 1) * 8, :, 0:W])
                    psm = pool_psmx.tile([128, CW], dt.float32, tag="psm")
                    mm = [(w1_s[:], shuf[:, 0:W], slice(0, W))]
                    if jc == 0:
                        mm.append((ones1_s[:], padrow_s[:, 0:W], slice(0, W)))
                    if j0 + W > mwin0:
                        a = max(j0, mwin0)
                        mm.append((selg_s[:, g, :],
                                   masks_s[:, s, a - mwin0:j0 + W - mwin0],
                                   slice(a - j0, W)))
                    for mi, (lh, rh, csl) in enumerate(mm):
                        nc.tensor.matmul(psm[:, csl], lh, rh,
                                         start=(mi == 0),
                                         stop=(mi == len(mm) - 1))
                    if g % 4 == 0:
                        dmc = pool_dmc.tile([128, 4, CW], dt.float32,
                                            tag="dmc")
                    nc.scalar.copy(dmc[:, g % 4, 0:W], psm[:, 0:W])
                    if g % 4 == 3:
                        nc.sync.dma_start(
                            dm_dram[g - 3:g + 1, :, j0:j0 + W]
                            .rearrange("g p j -> p g j"), dmc[:, :, 0:W])

            dm_s0 = pool_dram.tile([16, 128, 2176], dt.float32, tag="dm0")

            # ============ phase 1: QT, K^T (slot-0 jc interleaved), V ======
            with tc.tile_pool(name="ph1", bufs=1) as p1, \
                 tc.tile_pool(name="ph1c", bufs=3) as p1c, \
                 tc.tile_pool(name="ph1x", bufs=2) as p1x, \
                 tc.tile_pool(name="ph1q", bufs=2) as p1q, \
                 tc.tile_pool(name="ps1", bufs=2, space="PSUM") as ps1:
                wv_s = p1.tile([128, 8, DIM], dt.bfloat16)
                xq_hi_s = p1.tile([128, 8, NSLOT, 128], dt.bfloat16)
                xq_lo_s = p1.tile([128, 8, NSLOT, 128], dt.bfloat16)
                for c in range(8):
                    nc.sync.dma_start(wv_s[:, c, :], wv[:, c, :])
                nc.sync.dma_start(xq_hi_s[:], xqT_hi[:])
                nc.sync.dma_start(xq_lo_s[:], xqT_lo[:])
                for c in range(8):
                    nc.sync.dma_start(kt_hi_d[:, c, 0:128], memKT_hi[:, c, :])
                    nc.sync.dma_start(kt_lo_d[:, c, 0:128], memKT_lo[:, c, :])
                nc.sync.dma_start(v_d[:, :, 0, :], memV[:])

                # QT (wq streamed per m-chunk)
                for mc in range(8):
                    wqc_h = p1q.tile([128, 8, 128], dt.bfloat16, tag="wqch")
                    wqc_l = p1q.tile([128, 8, 128], dt.bfloat16, tag="wqcl")
                    nc.sync.dma_start(wqc_h[:],
                                      wq_hi[:, :, mc * 128:(mc + 1) * 128])
                    nc.sync.dma_start(wqc_l[:],
                                      wq_lo[:, :, mc * 128:(mc + 1) * 128])
                    for s in range(NSLOT):
                        psq = ps1.tile([128, 128], dt.float32, tag="psq")
                        for dc in range(8):
                            nc.tensor.matmul(psq[:], wqc_h[:, dc, :],
                                             xq_hi_s[:, dc, s, :],
                                             start=(dc == 0), stop=False)
                            nc.tensor.matmul(psq[:], wqc_h[:, dc, :],
                                             xq_lo_s[:, dc, s, :],
                                             start=False, stop=False)
                            nc.tensor.matmul(psq[:], wqc_l[:, dc, :],
                                             xq_hi_s[:, dc, s, :],
                                             start=False, stop=(dc == 7))
                        qt32 = p1q.tile([128, 128], dt.float32, tag="qt32")
                        nc.scalar.copy(qt32[:], psq[:])
                        nc.vector.tensor_copy(qt_hi[:, s, mc, :], qt32[:])
                        nc.vector.tensor_sub(qt_lo[:, s, mc, :], qt32[:],
                                             qt_hi[:, s, mc, :])

                def do_kt(njc):
                    xth = p1x.tile([128, 8, 512], dt.bfloat16, tag="xth")
                    xtl = p1x.tile([128, 8, 512], dt.bfloat16, tag="xtl")
                    nc.sync.dma_start(
                        xth[:], xT_hi[:, :, njc * 512:(njc + 1) * 512])
                    nc.sync.dma_start(
                        xtl[:], xT_lo[:, :, njc * 512:(njc + 1) * 512])
                    for ic in range(8):
                        wkc_h = p1q.tile([128, 8, 128], dt.bfloat16,
                                         tag="wkch")
                        wkc_l = p1q.tile([128, 8, 128], dt.bfloat16,
                                         tag="wkcl")
                        nc.sync.dma_start(
                            wkc_h[:], wk_hi[:, :, ic * 128:(ic + 1) * 128])
                        nc.sync.dma_start(
                            wkc_l[:], wk_lo[:, :, ic * 128:(ic + 1) * 128])
                        ps = ps1.tile([128, 512], dt.float32, tag="psk")
                        for dc in range(8):
                            nc.tensor.matmul(ps[:], wkc_h[:, dc, :],
                                             xth[:, dc, :], start=(dc == 0),
                                             stop=False)
                            nc.tensor.matmul(ps[:], wkc_h[:, dc, :],
                                             xtl[:, dc, :], start=False,
                                             stop=False)
                            nc.tensor.matmul(ps[:], wkc_l[:, dc, :],
                                             xth[:, dc, :], start=False,
                                             stop=(dc == 7))
                        khi = p1c.tile([128, 512], dt.bfloat16, tag="khi")
                        klo = p1c.tile([128, 512], dt.bfloat16, tag="klo")
                        nc.scalar.copy(khi[:], ps[:])
                        nc.vector.tensor_sub(klo[:], ps[:], khi[:])
                        cols = slice(128 + njc * 512, 128 + (njc + 1) * 512)
                        nc.sync.dma_start(kt_hi_d[:, ic, cols], khi[:])
                        nc.sync.dma_start(kt_lo_d[:, ic, cols], klo[:])

                do_kt(0)
                # slot-0 QK/mix pipeline interleaves with remaining phase-1
                for jc in range(3):
                    do_jc(0, SLOT_J[0], 3, 5, dm_s0, jc)
                for njc in range(1, 4):
                    do_kt(njc)

                for nb in range(16):
                    xthv = p1x.tile([128, 8, 128], dt.bfloat16, tag="xthv")
                    nc.sync.dma_start(
                        xthv[:], xT_hi[:, :, nb * 128:(nb + 1) * 128])
                    for fh in range(2):
                        ps = ps1.tile([128, 512], dt.float32, tag="psk")
                        for dc in range(8):
                            nc.tensor.matmul(
                                ps[:], xthv[:, dc, :],
                                wv_s[:, dc, fh * 512:(fh + 1) * 512],
                                start=(dc == 0), stop=(dc == 7))
                        vsb = p1c.tile([128, 512], dt.bfloat16, tag="vsb")
                        nc.scalar.copy(vsb[:], ps[:])
                        nc.sync.dma_start(
                            v_d[:, fh * 8:(fh + 1) * 8, 1 + nb, :], vsb[:])

            # ============ phase 2: attention =============================
            with tc.tile_pool(name="dmg", bufs=2) as pool_dmg, \
                 tc.tile_pool(name="tk1", bufs=1) as pool_tk1, \
                 tc.tile_pool(name="tk2", bufs=2) as pool_tk2, \
                 tc.tile_pool(name="sm", bufs=4) as pool_sm, \
                 tc.tile_pool(name="ptq", bufs=1) as pool_ptq, \
                 tc.tile_pool(name="vs", bufs=2) as pool_vs, \
                 tc.tile_pool(name="outs", bufs=1) as pool_out, \
                 tc.tile_pool(name="psPt", bufs=2, space="PSUM") as pool_pspt, \
                 tc.tile_pool(name="psAvO", bufs=2, space="PSUM") as pool_psav:

                def do_gq(s, J, NJC, NJP, dm_dram, av32, gq):
                    ptq = pool_ptq.tile([128, 17, 16, 32], dt.bfloat16,
                                        tag="ptq")
                    for gi in range(4):
                        g = gq * 4 + gi
                        dmg = pool_dmg.tile([128, 2176], dt.float32,
                                            tag="dmg")
                        nc.sync.dma_start(dmg[:, 0:J], dm_dram[g, :, 0:J])
                        scr = pool_tk1.tile([128, 2176], dt.float32,
                                            tag="scr")
                        cands = pool_tk2.tile([128, 64], dt.float32,
                                              tag="cands")
                        if s == 0:
                            nc.vector.max(cands[:, 0:8], dmg[:, 0:J])
                            nc.vector.match_replace(scr[:, 0:J],
                                                    cands[:, 0:8],
                                                    dmg[:, 0:J], NEG)
                            for r in range(1, 8):
                                nc.vector.max(cands[:, r * 8:(r + 1) * 8],
                                              scr[:, 0:J])
                                if r < 7:
                                    nc.vector.match_replace(
                                        scr[:, 0:J],
                                        cands[:, r * 8:(r + 1) * 8],
                                        scr[:, 0:J], NEG)
                        else:
                            RR = 4 if s == 1 else 3
                            segw = (J - 128) // 8
                            segs = [(0, 128)] + [
                                (128 + k * segw, 128 + (k + 1) * segw)
                                for k in range(8)]
                            cpool = pool_tk2.tile([128, 9 * 8 * 4],
                                                  dt.float32, tag="cpool")
                            for rr in range(RR):
                                src_t = dmg if rr == 0 else scr
                                for si2, (a2, b2) in enumerate(segs):
                                    nc.vector.max(
                                        cpool[:, (rr * 9 + si2) * 8:
                                              (rr * 9 + si2) * 8 + 8],
                                        src_t[:, a2:b2])
                                if rr < RR - 1:
                                    for si2, (a2, b2) in enumerate(segs):
                                        nc.vector.match_replace(
                                            scr[:, a2:b2],
                                            cpool[:, (rr * 9 + si2) * 8:
                                                  (rr * 9 + si2) * 8 + 8],
                                            src_t[:, a2:b2], NEG)
                            ncand = 9 * 8 * RR
                            nc.vector.max(cands[:, 0:8], cpool[:, 0:ncand])
                            nc.vector.match_replace(cpool[:, 0:ncand],
                                                    cands[:, 0:8],
                                                    cpool[:, 0:ncand], NEG)
                            for r in range(1, 8):
                                nc.vector.max(cands[:, r * 8:(r + 1) * 8],
                                              cpool[:, 0:ncand])
                                if r < 7:
                                    nc.vector.match_replace(
                                        cpool[:, 0:ncand],
                                        cands[:, r * 8:(r + 1) * 8],
                                        cpool[:, 0:ncand], NEG)
                        negm = pool_sm.tile([128, 1], dt.float32, tag="negm")
                        nc.vector.tensor_scalar_mul(negm[:], cands[:, 0:1],
                                                    -1.0)
                        ec = pool_sm.tile([128, 64], dt.float32, tag="ec")
                        nc.scalar.activation(ec[:], cands[:],
                                             mybir.ActivationFunctionType.Exp,
                                             bias=negm[:])
                        den = pool_sm.tile([128, 1], dt.float32, tag="den")
                        nc.vector.reduce_sum(den[:], ec[:],
                                             axis=mybir.AxisListType.X)
                        rden = pool_sm.tile([128, 1], dt.float32, tag="rden")
                        nc.vector.reciprocal(rden[:], den[:])
                        p2 = pool_tk2.tile([128, 2176], dt.float32, tag="p2")
                        nc.scalar.activation(p2[:, 0:J], dmg[:, 0:J],
                                             mybir.ActivationFunctionType.Exp,
                                             bias=negm[:])
                        nc.vector.tensor_scalar(scr[:, 0:J], dmg[:, 0:J],
                                                cands[:, 63:64], rden[:],
                                                mybir.AluOpType.is_ge,
                                                mybir.AluOpType.mult)
                        ptil = pool_tk2.tile([128, 2176], dt.bfloat16,
                                             tag="ptil")
                        nc.gpsimd.tensor_mul(ptil[:, 0:J], scr[:, 0:J],
                                             p2[:, 0:J])
                        for jp4 in range((NJP + 3) // 4):
                            nq = min(4, NJP - jp4 * 4)
                            pspt = pool_pspt.tile([128, 512], dt.float32,
                                                  tag="pspt")
                            for q in range(nq):
                                jp = jp4 * 4 + q
                                nc.tensor.matmul(
                                    pspt[:, q * 128:(q + 1) * 128],
                                    ptil[:, jp * 128:(jp + 1) * 128],
                                    w2_s[:], start=True, stop=True)
                            dst = ptq[:, jp4 * 4:jp4 * 4 + nq, :,
                                      gi * 8:(gi + 1) * 8]
                            nc.scalar.copy(dst[:], pspt[:, 0:nq * 128])
                    # AV over this quad (32 query columns)
                    psav = pool_psav.tile([128, 8, 32], dt.float32,
                                          tag="avo")
                    for ko in range(16):
                        vst = pool_vs.tile([128, 17, 64], dt.bfloat16,
                                           tag="vst")
                        nc.gpsimd.dma_start(vst[:, 0:NJP, :],
                                            v_d[:, ko, 0:NJP, :])
                        pb = (ko % 2) * 64
                        for jp in range(NJP):
                            nc.tensor.matmul(
                                psav[pb:pb + 64, ko // 2, :],
                                vst[:, jp, :], ptq[:, jp, ko, :],
                                start=(jp == 0), stop=(jp == NJP - 1))
                    nc.vector.tensor_copy(av32[:, :, gq * 32:(gq + 1) * 32],
                                          psav[:])

                def do_tail(s, J, av32):
                    av_sb = pool_out.tile([128, 8, 128], dt.bfloat16,
                                          tag="av")
                    nc.vector.tensor_copy(av_sb[:], av32[:])
                    osb = pool_out.tile([128, DIM], dt.float32, tag="osb")
                    for fh in range(2):
                        pso = pool_psav.tile([128, 512], dt.float32,
                                             tag="avo")
                        for cp in range(8):
                            nc.tensor.matmul(
                                pso[:], av_sb[:, cp, :],
                                wo_s[:, cp, fh * 512:(fh + 1) * 512],
                                start=(cp == 0), stop=(cp == 7))
                        nc.vector.tensor_add(osb[:, fh * 512:(fh + 1) * 512],
                                             pso[:],
                                             bo_s[:, fh * 512:(fh + 1) * 512])
                    nc.sync.dma_start(out_d[s, :, :], osb[:])

                for s in range(NSLOT):
                    J = SLOT_J[s]
                    NJC = (J - 128) // CW + 1
                    NJP = J // 128
                    if s == 0:
                        dm_dram = dm_s0
                    else:
                        dm_dram = pool_dram.tile([16, 128, 2176], dt.float32,
                                                 tag=f"dm{s % 2}")
                        for jc in range(NJC):
                            do_jc(s, J, NJC, NJP, dm_dram, jc)
                    av32 = pool_out.tile([128, 8, 128], dt.float32,
                                         tag="av32")
                    for gq in range(4):
                        do_gq(s, J, NJC, NJP, dm_dram, av32, gq)
                    do_tail(s, J, av32)

    nc.finalize()
    return nc


_prep_cache = {}


def _host_prep(core, inputs):
    x = np.asarray(inputs["x"], dtype=np.float32)
    Wq = np.asarray(inputs["Wq"], dtype=np.float32) * (D ** -0.5)
    Wk = np.asarray(inputs["Wk"], dtype=np.float32)
    Wv = np.asarray(inputs["Wv"], dtype=np.float32)
    Wo = np.asarray(inputs["Wo"], dtype=np.float32)
    bo = np.asarray(inputs["bo"], dtype=np.float32)
    pre = np.asarray(inputs["pre_proj"], dtype=np.float32)
    post = np.asarray(inputs["post_proj"], dtype=np.float32)
    mem_k = np.asarray(inputs["mem_k"], dtype=np.float32)
    mem_v = np.asarray(inputs["mem_v"], dtype=np.float32)

    b = core // 4
    g = core % 4
    qbs = QBS_OF_G[g]

    xb = x[b]
    if ("xT", b) not in _prep_cache:
        xT = np.ascontiguousarray(xb.T)
        _prep_cache[("xT", b)] = _split_hi_lo(xT)
    xT_hi, xT_lo = _prep_cache[("xT", b)]
    xq = np.concatenate([xb[qb * 128:(qb + 1) * 128] for qb in qbs], axis=0)
    xqT = np.ascontiguousarray(xq.T)                      # [DIM, 512]
    xqT_hi, xqT_lo = _split_hi_lo(xqT)

    if "w" not in _prep_cache:
        _prep_cache["w"] = (_split_hi_lo(Wq), _split_hi_lo(Wk))
    (wq_hi, wq_lo), (wk_hi, wk_lo) = _prep_cache["w"]

    mkt = np.zeros((128, 8, 128), dtype=np.float32)
    for h in range(H):
        mkt[(h % 2) * 64:(h % 2) * 64 + 64, h // 2, 0:M] = mem_k[h].T
    mkt_hi, mkt_lo = _split_hi_lo(mkt)
    mv = np.zeros((128, DIM), dtype=np.float32)
    mv[0:M] = mem_v.transpose(1, 0, 2).reshape(M, DIM)

    w1 = np.zeros((128, 128), dtype=np.float32)
    for isub in range(8):
        for h in range(H):
            for k in range(H):
                w1[isub * 16 + h, k * 8 + isub] = pre[h, k]
    w2 = np.zeros((128, 128), dtype=np.float32)
    for isub in range(8):
        for k in range(H):
            for ko in range(H):
                w2[k * 8 + isub, ko * 8 + isub] = post[k, ko]
    selg = np.zeros((128, 16, 128), dtype=np.float32)
    for gg in range(16):
        for isub in range(8):
            selg[gg * 8 + isub, gg, np.arange(16) * 8 + isub] = 1.0
    ones1 = np.ones((1, 128), dtype=np.float32)
    padrow = np.zeros((1, CW), dtype=np.float32)
    padrow[0, M:128] = NEG

    masks = np.zeros((128, NSLOT, 640), dtype=np.float32)
    for si, qb in enumerate(qbs):
        J = SLOT_J[si]
        base = J - 640
        for gg in range(16):
            for isub in range(8):
                i_glob = qb * 128 + gg * 8 + isub
                jmax = 128 + i_glob + 1
                cols = np.arange(base, J)
                masks[gg * 8 + isub, si, cols >= jmax] = NEG

    wo_r = np.zeros((128, 8, DIM), dtype=np.float32)
    for ko in range(H):
        wo_r[(ko % 2) * 64:(ko % 2) * 64 + 64, ko // 2, :] = \
            Wo[ko * 64:(ko + 1) * 64, :]

    f = np.ascontiguousarray
    return {
        "xT_hi": f(_chunk_part(xT_hi)), "xT_lo": f(_chunk_part(xT_lo)),
        "xqT_hi": f(_chunk_part(xqT_hi).reshape(128, 8, NSLOT, 128)),
        "xqT_lo": f(_chunk_part(xqT_lo).reshape(128, 8, NSLOT, 128)),
        "wq_hi": f(_chunk_part(wq_hi)), "wq_lo": f(_chunk_part(wq_lo)),
        "wk_hi": f(_chunk_part(wk_hi)), "wk_lo": f(_chunk_part(wk_lo)),
        "wv": f(_chunk_part(Wv.astype(bf16))),
        "wo": f(wo_r.astype(bf16)),
        "memKT_hi": f(mkt_hi), "memKT_lo": f(mkt_lo),
        "memV": f(mv.astype(bf16)),
        "w1": w1, "w2": f(w2.astype(bf16)), "selg": selg, "ones1": ones1,
        "padrow": padrow, "masks": masks,
        "bo_in": f(np.broadcast_to(bo[None, :], (128, DIM)).copy()),
    }


def kernel(**inputs) -> np.ndarray:
    from concourse.bass_utils import run_bass_kernel_spmd

    _prep_cache.clear()

    if "nc" not in _nc_cache:
        _nc_cache["nc"] = build_nc()
    nc = _nc_cache["nc"]

    in_maps = [_host_prep(c, inputs) for c in range(8)]
    res = run_bass_kernel_spmd(nc, in_maps, core_ids=list(range(8)))

    out = np.zeros((B, N, DIM), dtype=np.float32)
    for c in range(8):
        b = c // 4
        qbs = QBS_OF_G[c % 4]
        o = res.results[c]["out"]
        for si, qb in enumerate(qbs):
            out[b, qb * 128:(qb + 1) * 128, :] = o[si]
    return out
